# revision 10
# baseline (speedup 1.0000x reference)
"""Trainium2 Bass kernel for multi-head attention (B=2, S=2048, D=1024, H=16).

Sharding: 8 cores = 2 (batch, data-parallel) x 4 (head-groups, tensor-parallel).
Each core (b, g) handles batch b and heads [4g, 4g+4) (a 256-wide slice of the
model dim), computing a partial output contribution; the host sums the 4
head-group partials per batch and adds the output bias.

Per-core pipeline (all matmuls on the PE array):
  phase 1: qp^T, kp^T  (feature-major, [d=256, s=2048]) via W^T-stationary
           matmuls in fp32r; vp (sequence-major, [s=2048, 260]) with a
           fused bias+ones augmentation column per head (ones column later
           produces the softmax row-sums for free during attn@V).
  phase 2: per (head-pair, q-block): scores^T = kp^T.T-packed matmuls
           (two heads row-packed on the 128-row array, K=64 each), exp on
           the scalar engine directly out of PSUM in 3-bank groups,
           attn@V accumulated in PSUM with M=65 (64 dims + rowsum).
  phase 3: normalize by the rowsum (reciprocal + gpsimd partition
           broadcast + vector multiply) and apply the output projection,
           streaming partials to DRAM.
"""

import os
import numpy as np
import ml_dtypes

import concourse.bass as bass
import concourse.bacc as bacc
import concourse.mybir as mybir
import concourse.tile as tile
from concourse.bass_utils import run_bass_kernel_spmd

F32 = mybir.dt.float32
F32R = mybir.dt.float32r
BF16 = mybir.dt.bfloat16
AF = mybir.ActivationFunctionType

B, S, D = 2, 2048, 1024
H, DK = 16, 64
G = 4                  # head-groups (tensor parallel across cores)
DG = D // G            # 256 features per core
HPG = H // G           # 4 heads per core (2 row-packed pairs)
VEXT = HPG * (DK + 1)  # 260: per head [64 vp dims | 1 ones column]
P = 128
N_CORES = 8

_NC = None


def _build_program():
    nc = bacc.Bacc("TRN2", target_bir_lowering=False)
    qT = nc.dram_tensor("qT", [D, S], F32R, kind="ExternalInput")
    kT = nc.dram_tensor("kT", [D, S], F32R, kind="ExternalInput")
    vT = nc.dram_tensor("vT", [D, S], F32R, kind="ExternalInput")
    wqT = nc.dram_tensor("wqT", [D, DG], F32R, kind="ExternalInput")
    wkT = nc.dram_tensor("wkT", [D, DG], F32R, kind="ExternalInput")
    wvm = nc.dram_tensor("wvm", [D, VEXT], F32R, kind="ExternalInput")
    wvb = nc.dram_tensor("wvb", [1, VEXT], F32R, kind="ExternalInput")
    onesr = nc.dram_tensor("onesr", [1, P], F32R, kind="ExternalInput")
    woT = nc.dram_tensor("woT", [DG, D], BF16, kind="ExternalInput")
    bqv = nc.dram_tensor("bqv", [DG], F32, kind="ExternalInput")
    bkv = nc.dram_tensor("bkv", [DG], F32, kind="ExternalInput")
    out = nc.dram_tensor("out", [S, D], F32, kind="ExternalOutput")

    with tile.TileContext(nc) as tc:
        _body(nc, tc, qT, kT, vT, wqT, wkT, wvm, wvb, onesr, woT, bqv, bkv, out)
    nc.compile()
    return nc


def _body(nc, tc, qT, kT, vT, wqT, wkT, wvm, wvb, onesr, woT, bqv, bkv, out):
    with (
        tc.tile_pool(name="consts", bufs=1) as consts,
        tc.tile_pool(name="persist", bufs=1) as persist,
        tc.tile_pool(name="stage", bufs=2) as stage,
        tc.tile_pool(name="etp", bufs=3) as etp,
        tc.tile_pool(name="small", bufs=2) as small,
        tc.tile_pool(name="outp", bufs=3) as outp,
        tc.tile_pool(name="psA", bufs=2, space="PSUM") as psA,
        tc.tile_pool(name="psG", bufs=2, space="PSUM") as psG,
    ):
        # --- constants / weights ---
        wq_sb = consts.tile([P, 8, DG], F32R)
        nc.sync.dma_start(wq_sb[:], wqT[:].rearrange("(t p) m -> p t m", p=P))
        wk_sb = consts.tile([P, 8, DG], F32R)
        nc.sync.dma_start(wk_sb[:], wkT[:].rearrange("(t p) m -> p t m", p=P))
        wv_sb = consts.tile([P, 8, VEXT], F32R)
        nc.sync.dma_start(wv_sb[:], wvm[:].rearrange("(t p) m -> p t m", p=P))
        wvb_sb = consts.tile([1, VEXT], F32R)
        nc.sync.dma_start(wvb_sb[:], wvb[:])
        wo_sb = consts.tile([P, 2, D], BF16)
        nc.sync.dma_start(wo_sb[:], woT[:].rearrange("(t p) o -> p t o", p=P))
        bq_sb = consts.tile([P, 2], F32)
        nc.sync.dma_start(bq_sb[:], bqv[:].rearrange("(t p) -> p t", p=P))
        bk_sb = consts.tile([P, 2], F32)
        nc.sync.dma_start(bk_sb[:], bkv[:].rearrange("(t p) -> p t", p=P))
        ones_sb = consts.tile([1, P], F32R)
        nc.sync.dma_start(ones_sb[:], onesr[:])

        # warm the ACT exp table early so the ~2.7us load overlaps phase 1
        warm = consts.tile([1, 8], F32)
        nc.vector.memset(warm[:], 0.0)
        nc.scalar.activation(warm[:], warm[:], AF.Exp)

        # --- persistent activations ---
        qpT_sb = persist.tile([P, 2, S], F32R)   # [d%128, d-tile(=pair), s]
        kpT_sb = persist.tile([P, 2, S], F32R)
        vp_sb = persist.tile([P, 16, VEXT], BF16)  # [s%128, s-tile, 4*(64+1)]
        an_sb = persist.tile([P, 2, S], BF16)   # normalized attn output^T

        # --- phase 1a: qp^T / kp^T projections (fp32r, W^T stationary) ---
        for src, w_sb, b_sb, dst in (
            (qT, wq_sb, bq_sb, qpT_sb),
            (kT, wk_sb, bk_sb, kpT_sb),
        ):
            for j in range(4):  # s-blocks of 512
                xb = stage.tile([P, 8, 512], F32R, tag="xb")
                nc.sync.dma_start(
                    xb[:],
                    src[:].rearrange("(t p) s -> p t s", p=P)[
                        :, :, j * 512 : (j + 1) * 512
                    ],
                )
                for dt in range(2):
                    ps = psA.tile([P, 512], F32, tag="a", name="ps_proj")
                    for kt in range(8):
                        nc.tensor.matmul(
                            ps[:],
                            lhsT=w_sb[:, kt, dt * P : (dt + 1) * P],
                            rhs=xb[:, kt, :],
                            start=(kt == 0),
                            stop=(kt == 7),
                        )
                    nc.vector.tensor_scalar_add(
                        dst[:, dt, j * 512 : (j + 1) * 512], ps[:], b_sb[:, dt : dt + 1]
                    )

        # --- phase 1b: vp (sequence-major) with fused bias+ones row ---
        for st in range(16):
            vtb = stage.tile([P, 8, P], F32R, tag="vtb")
            nc.sync.dma_start(
                vtb[:],
                vT[:].rearrange("(t p) s -> p t s", p=P)[:, :, st * P : (st + 1) * P],
            )
            psv = psA.tile([P, VEXT], F32, tag="a", name="ps_v")
            for kt in range(8):
                nc.tensor.matmul(
                    psv[:],
                    lhsT=vtb[:, kt, :],
                    rhs=wv_sb[:, kt, :],
                    start=(kt == 0),
                    stop=False,
                )
            nc.tensor.matmul(
                psv[:],
                lhsT=ones_sb[:],
                rhs=wvb_sb[:],
                start=False,
                stop=True,
            )
            nc.vector.tensor_copy(vp_sb[:, st, :], psv[:])

        # --- phase 2+3 per q-block ---
        GRP = 3  # PSUM banks per exp group
        for qb in range(4):
            qs = slice(qb * 512, (qb + 1) * 512)
            for pair in range(2):
                c_ps = [
                    psA.tile([DK + 1, 512], F32, tag="a", name=f"c{hh}")
                    for hh in range(2)
                ]
                tiles = [(kt, hh) for kt in range(16) for hh in range(2)]
                for g0 in range(0, len(tiles), GRP):
                    grp = tiles[g0 : g0 + GRP]
                    gps = psG.tile([P, GRP * 512], F32, tag="g", name="gps")
                    for i, (kt, hh) in enumerate(grp):
                        hp = slice(hh * DK, (hh + 1) * DK)
                        nc.tensor.matmul(
                            gps[:, i * 512 : (i + 1) * 512],
                            lhsT=kpT_sb[hp, pair, kt * P : (kt + 1) * P],
                            rhs=qpT_sb[hp, pair, qs],
                            start=True,
                            stop=True,
                        )
                    et = etp.tile([P, GRP * 512], BF16, tag="e", name="et")
                    w = len(grp) * 512
                    nc.scalar.activation(
                        et[:, :w], gps[:, :w], AF.Exp, scale=1.0 / np.sqrt(DK)
                    )
                    for i, (kt, hh) in enumerate(grp):
                        h = 2 * pair + hh
                        nc.tensor.matmul(
                            c_ps[hh][:],
                            lhsT=vp_sb[:, kt, h * (DK + 1) : (h + 1) * (DK + 1)],
                            rhs=et[:, i * 512 : (i + 1) * 512],
                            start=(kt == 0),
                            stop=(kt == 15),
                        )
                # normalize: divide by the rowsum (row 64 of each accumulator)
                for hh in range(2):
                    rinv = small.tile([1, 512], F32, tag="rinv")
                    nc.vector.reciprocal(rinv[:], c_ps[hh][DK : DK + 1, :])
                    rbc = small.tile([DK, 512], F32, tag="rbc")
                    nc.gpsimd.partition_broadcast(rbc[:], rinv[:])
                    nc.vector.tensor_tensor(
                        an_sb[hh * DK : (hh + 1) * DK, pair, qs],
                        c_ps[hh][:DK, :],
                        rbc[:],
                        mybir.AluOpType.mult,
                    )
            # output projection for this q-block
            for qt in range(4):
                q0 = qb * 512 + qt * P
                for o in range(2):
                    dps = psA.tile([P, 512], F32, tag="a", name="dps")
                    for p2 in range(2):
                        nc.tensor.matmul(
                            dps[:],
                            lhsT=an_sb[:, p2, q0 : q0 + P],
                            rhs=wo_sb[:, p2, o * 512 : (o + 1) * 512],
                            start=(p2 == 0),
                            stop=(p2 == 1),
                        )
                    osb = outp.tile([P, 512], F32, tag="o")
                    nc.vector.tensor_copy(osb[:], dps[:])
                    nc.sync.dma_start(out[q0 : q0 + P, o * 512 : (o + 1) * 512], osb[:])


def _get_program():
    global _NC
    if _NC is None:
        _NC = _build_program()
    return _NC


def _make_in_maps(v, k, q, Wv, bv, Wk, bk, Wq, bq, Wo, bo):
    f32 = np.float32
    qT = [np.ascontiguousarray(q[b].T, dtype=f32) for b in range(B)]
    kT = [np.ascontiguousarray(k[b].T, dtype=f32) for b in range(B)]
    vT = [np.ascontiguousarray(v[b].T, dtype=f32) for b in range(B)]

    per_group = []
    for g in range(G):
        gs = slice(g * DG, (g + 1) * DG)
        wqT = np.ascontiguousarray(Wq[gs, :].T, dtype=f32)
        wkT = np.ascontiguousarray(Wk[gs, :].T, dtype=f32)
        wvm = np.zeros((D, VEXT), dtype=f32)
        wvb = np.zeros((1, VEXT), dtype=f32)
        for h in range(HPG):
            cs = slice(h * (DK + 1), h * (DK + 1) + DK)
            rows = slice(g * DG + h * DK, g * DG + (h + 1) * DK)
            wvm[:, cs] = Wv[rows, :].T
            wvb[0, cs] = bv[rows]
            wvb[0, h * (DK + 1) + DK] = 1.0
        woT = np.ascontiguousarray(Wo[:, gs].T).astype(ml_dtypes.bfloat16)
        per_group.append(
            dict(
                wqT=wqT,
                wkT=wkT,
                wvm=wvm,
                wvb=wvb,
                woT=woT,
                bqv=np.ascontiguousarray(bq[gs], dtype=f32),
                bkv=np.ascontiguousarray(bk[gs], dtype=f32),
            )
        )

    in_maps = []
    for c in range(N_CORES):
        b, g = c // G, c % G
        m = dict(qT=qT[b], kT=kT[b], vT=vT[b],
                 onesr=np.ones((1, P), dtype=f32), **per_group[g])
        in_maps.append(m)
    return in_maps


def _gather(results, bo):
    out = np.zeros((B, S, D), dtype=np.float32)
    for c in range(N_CORES):
        b = c // G
        out[b] += results[c]["out"]
    out += bo.astype(np.float32)
    return out


def run(v, k, q, Wv, bv, Wk, bk, Wq, bq, Wo, bo, trace=False):
    nc = _get_program()
    in_maps = _make_in_maps(v, k, q, Wv, bv, Wk, bk, Wq, bq, Wo, bo)
    res = run_bass_kernel_spmd(
        nc, in_maps, core_ids=list(range(N_CORES)), trace=trace
    )
    return _gather(res.results, np.asarray(bo)), res


def kernel(v, k, q, Wv, bv, Wk, bk, Wq, bq, Wo, bo):
    args = [np.asarray(x, dtype=np.float32)
            for x in (v, k, q, Wv, bv, Wk, bk, Wq, bq, Wo, bo)]
    out, _ = run(*args, trace=bool(int(os.environ.get("MHA_TRACE", "0"))))
    return out


# revision 12
# speedup vs baseline: 1.2853x; 1.2853x over previous
"""Trainium2 Bass kernel for multi-head attention (B=2, S=2048, D=1024, H=16).

Sharding: 8 cores = 2 (batch, data-parallel) x 4 (head-groups, tensor-parallel).
Each core (b, g) handles batch b and heads [4g, 4g+4) (a 256-wide slice of the
model dim), computing a partial output contribution; the host sums the 4
head-group partials per batch and adds the output bias.

Per-core pipeline (all matmuls on the PE array):
  phase 1: qp^T, kp^T  (feature-major, [d=256, s=2048]) via W^T-stationary
           matmuls in fp32r; vp (sequence-major, [s=2048, 260]) with a
           fused bias+ones augmentation column per head (ones column later
           produces the softmax row-sums for free during attn@V).
  phase 2: per (head-pair, q-block): scores^T = kp^T.T-packed matmuls
           (two heads row-packed on the 128-row array, K=64 each), exp on
           the scalar engine directly out of PSUM in 3-bank groups,
           attn@V accumulated in PSUM with M=65 (64 dims + rowsum).
  phase 3: normalize by the rowsum (reciprocal + gpsimd partition
           broadcast + vector multiply) and apply the output projection,
           streaming partials to DRAM.
"""

import os
import numpy as np
import ml_dtypes

import concourse.bass as bass
import concourse.bacc as bacc
import concourse.mybir as mybir
import concourse.tile as tile
from concourse.bass_utils import run_bass_kernel_spmd

F32 = mybir.dt.float32
F32R = mybir.dt.float32r
BF16 = mybir.dt.bfloat16
AF = mybir.ActivationFunctionType

B, S, D = 2, 2048, 1024
H, DK = 16, 64
G = 4                  # head-groups (tensor parallel across cores)
DG = D // G            # 256 features per core
HPG = H // G           # 4 heads per core (2 row-packed pairs)
VEXT = HPG * (DK + 1)  # 260: per head [64 vp dims | 1 ones column]
P = 128
N_CORES = 8

_NC = None


def _build_program():
    nc = bacc.Bacc("TRN2", target_bir_lowering=False)
    qT = nc.dram_tensor("qT", [D, S], BF16, kind="ExternalInput")
    kT = nc.dram_tensor("kT", [D, S], BF16, kind="ExternalInput")
    vT = nc.dram_tensor("vT", [D, S], BF16, kind="ExternalInput")
    wqT = nc.dram_tensor("wqT", [D, DG], BF16, kind="ExternalInput")
    wkT = nc.dram_tensor("wkT", [D, DG], BF16, kind="ExternalInput")
    wvm = nc.dram_tensor("wvm", [D, VEXT], BF16, kind="ExternalInput")
    wvb = nc.dram_tensor("wvb", [1, VEXT], BF16, kind="ExternalInput")
    onesr = nc.dram_tensor("onesr", [1, P], BF16, kind="ExternalInput")
    woT = nc.dram_tensor("woT", [DG, D], BF16, kind="ExternalInput")
    bqv = nc.dram_tensor("bqv", [DG], F32, kind="ExternalInput")
    bkv = nc.dram_tensor("bkv", [DG], F32, kind="ExternalInput")
    out = nc.dram_tensor("out", [S, D], F32, kind="ExternalOutput")

    with tile.TileContext(nc) as tc:
        _body(nc, tc, qT, kT, vT, wqT, wkT, wvm, wvb, onesr, woT, bqv, bkv, out)
    nc.compile()
    return nc


def _body(nc, tc, qT, kT, vT, wqT, wkT, wvm, wvb, onesr, woT, bqv, bkv, out):
    with (
        tc.tile_pool(name="consts", bufs=1) as consts,
        tc.tile_pool(name="persist", bufs=1) as persist,
        tc.tile_pool(name="stage", bufs=2) as stage,
        tc.tile_pool(name="etp", bufs=3) as etp,
        tc.tile_pool(name="small", bufs=2) as small,
        tc.tile_pool(name="outp", bufs=3) as outp,
        tc.tile_pool(name="psA", bufs=2, space="PSUM") as psA,
        tc.tile_pool(name="psG", bufs=2, space="PSUM") as psG,
    ):
        # --- constants / weights ---
        wq_sb = consts.tile([P, 8, DG], BF16)
        nc.sync.dma_start(wq_sb[:], wqT[:].rearrange("(t p) m -> p t m", p=P))
        wk_sb = consts.tile([P, 8, DG], BF16)
        nc.sync.dma_start(wk_sb[:], wkT[:].rearrange("(t p) m -> p t m", p=P))
        wv_sb = consts.tile([P, 8, VEXT], BF16)
        nc.sync.dma_start(wv_sb[:], wvm[:].rearrange("(t p) m -> p t m", p=P))
        wvb_sb = consts.tile([1, VEXT], BF16)
        nc.sync.dma_start(wvb_sb[:], wvb[:])
        wo_sb = consts.tile([P, 2, D], BF16)
        nc.sync.dma_start(wo_sb[:], woT[:].rearrange("(t p) o -> p t o", p=P))
        bq_sb = consts.tile([P, 2], F32)
        nc.sync.dma_start(bq_sb[:], bqv[:].rearrange("(t p) -> p t", p=P))
        bk_sb = consts.tile([P, 2], F32)
        nc.sync.dma_start(bk_sb[:], bkv[:].rearrange("(t p) -> p t", p=P))
        ones_sb = consts.tile([1, P], BF16)
        nc.sync.dma_start(ones_sb[:], onesr[:])

        # warm the ACT exp table early so the ~2.7us load overlaps phase 1
        warm = consts.tile([1, 8], F32)
        nc.vector.memset(warm[:], 0.0)
        nc.scalar.activation(warm[:], warm[:], AF.Exp)

        # --- persistent activations ---
        qpT_sb = persist.tile([P, 2, S], BF16)   # [d%128, d-tile(=pair), s]
        kpT_sb = persist.tile([P, 2, S], BF16)
        vp_sb = persist.tile([P, 16, VEXT], BF16)  # [s%128, s-tile, 4*(64+1)]
        an_sb = persist.tile([P, 2, S], BF16)   # normalized attn output^T

        # --- phase 1a: qp^T / kp^T projections (fp32r, W^T stationary) ---
        for src, w_sb, b_sb, dst in (
            (qT, wq_sb, bq_sb, qpT_sb),
            (kT, wk_sb, bk_sb, kpT_sb),
        ):
            for j in range(4):  # s-blocks of 512
                xb = stage.tile([P, 8, 512], BF16, tag="xb")
                nc.sync.dma_start(
                    xb[:],
                    src[:].rearrange("(t p) s -> p t s", p=P)[
                        :, :, j * 512 : (j + 1) * 512
                    ],
                )
                for dt in range(2):
                    ps = psA.tile([P, 512], F32, tag="a", name="ps_proj")
                    for kt in range(8):
                        nc.tensor.matmul(
                            ps[:],
                            lhsT=w_sb[:, kt, dt * P : (dt + 1) * P],
                            rhs=xb[:, kt, :],
                            start=(kt == 0),
                            stop=(kt == 7),
                        )
                    nc.vector.tensor_scalar_add(
                        dst[:, dt, j * 512 : (j + 1) * 512], ps[:], b_sb[:, dt : dt + 1]
                    )

        # --- phase 1b: vp (sequence-major) with fused bias+ones row ---
        for st in range(16):
            vtb = stage.tile([P, 8, P], BF16, tag="vtb")
            nc.sync.dma_start(
                vtb[:],
                vT[:].rearrange("(t p) s -> p t s", p=P)[:, :, st * P : (st + 1) * P],
            )
            psv = psA.tile([P, VEXT], F32, tag="a", name="ps_v")
            for kt in range(8):
                nc.tensor.matmul(
                    psv[:],
                    lhsT=vtb[:, kt, :],
                    rhs=wv_sb[:, kt, :],
                    start=(kt == 0),
                    stop=False,
                )
            nc.tensor.matmul(
                psv[:],
                lhsT=ones_sb[:],
                rhs=wvb_sb[:],
                start=False,
                stop=True,
            )
            nc.vector.tensor_copy(vp_sb[:, st, :], psv[:])

        # --- phase 2+3 per q-block ---
        GRP = 3  # PSUM banks per exp group
        for qb in range(4):
            qs = slice(qb * 512, (qb + 1) * 512)
            for pair in range(2):
                c_ps = [
                    psA.tile([DK + 1, 512], F32, tag="a", name=f"c{hh}")
                    for hh in range(2)
                ]
                tiles = [(kt, hh) for kt in range(16) for hh in range(2)]
                for g0 in range(0, len(tiles), GRP):
                    grp = tiles[g0 : g0 + GRP]
                    gps = psG.tile([P, GRP * 512], F32, tag="g", name="gps")
                    for i, (kt, hh) in enumerate(grp):
                        hp = slice(hh * DK, (hh + 1) * DK)
                        nc.tensor.matmul(
                            gps[:, i * 512 : (i + 1) * 512],
                            lhsT=kpT_sb[hp, pair, kt * P : (kt + 1) * P],
                            rhs=qpT_sb[hp, pair, qs],
                            start=True,
                            stop=True,
                        )
                    et = etp.tile([P, GRP * 512], BF16, tag="e", name="et")
                    w = len(grp) * 512
                    nc.scalar.activation(
                        et[:, :w], gps[:, :w], AF.Exp, scale=1.0 / np.sqrt(DK)
                    )
                    for i, (kt, hh) in enumerate(grp):
                        h = 2 * pair + hh
                        nc.tensor.matmul(
                            c_ps[hh][:],
                            lhsT=vp_sb[:, kt, h * (DK + 1) : (h + 1) * (DK + 1)],
                            rhs=et[:, i * 512 : (i + 1) * 512],
                            start=(kt == 0),
                            stop=(kt == 15),
                        )
                # normalize: divide by the rowsum (row 64 of each accumulator)
                for hh in range(2):
                    rsum = small.tile([1, 512], F32, tag="rsum")
                    nc.vector.tensor_copy(rsum[:], c_ps[hh][DK : DK + 1, :])
                    rinv = small.tile([1, 512], F32, tag="rinv")
                    nc.vector.reciprocal_approx_fast(rinv[:], rsum[:])
                    rbc = small.tile([DK, 512], F32, tag="rbc")
                    nc.gpsimd.partition_broadcast(rbc[:], rinv[:])
                    nc.vector.tensor_tensor(
                        an_sb[hh * DK : (hh + 1) * DK, pair, qs],
                        c_ps[hh][:DK, :],
                        rbc[:],
                        mybir.AluOpType.mult,
                    )
            # output projection for this q-block
            for qt in range(4):
                q0 = qb * 512 + qt * P
                for o in range(2):
                    dps = psA.tile([P, 512], F32, tag="a", name="dps")
                    for p2 in range(2):
                        nc.tensor.matmul(
                            dps[:],
                            lhsT=an_sb[:, p2, q0 : q0 + P],
                            rhs=wo_sb[:, p2, o * 512 : (o + 1) * 512],
                            start=(p2 == 0),
                            stop=(p2 == 1),
                        )
                    osb = outp.tile([P, 512], F32, tag="o")
                    nc.vector.tensor_copy(osb[:], dps[:])
                    nc.sync.dma_start(out[q0 : q0 + P, o * 512 : (o + 1) * 512], osb[:])


def _get_program():
    global _NC
    if _NC is None:
        _NC = _build_program()
    return _NC


def _make_in_maps(v, k, q, Wv, bv, Wk, bk, Wq, bq, Wo, bo):
    f32 = np.float32
    bf16 = ml_dtypes.bfloat16
    qT = [np.ascontiguousarray(q[b].T).astype(bf16) for b in range(B)]
    kT = [np.ascontiguousarray(k[b].T).astype(bf16) for b in range(B)]
    vT = [np.ascontiguousarray(v[b].T).astype(bf16) for b in range(B)]

    per_group = []
    for g in range(G):
        gs = slice(g * DG, (g + 1) * DG)
        wqT = np.ascontiguousarray(Wq[gs, :].T).astype(bf16)
        wkT = np.ascontiguousarray(Wk[gs, :].T).astype(bf16)
        wvm = np.zeros((D, VEXT), dtype=f32)
        wvb = np.zeros((1, VEXT), dtype=f32)
        for h in range(HPG):
            cs = slice(h * (DK + 1), h * (DK + 1) + DK)
            rows = slice(g * DG + h * DK, g * DG + (h + 1) * DK)
            wvm[:, cs] = Wv[rows, :].T
            wvb[0, cs] = bv[rows]
            wvb[0, h * (DK + 1) + DK] = 1.0
        wvm = wvm.astype(bf16)
        wvb = wvb.astype(bf16)
        woT = np.ascontiguousarray(Wo[:, gs].T).astype(bf16)
        per_group.append(
            dict(
                wqT=wqT,
                wkT=wkT,
                wvm=wvm,
                wvb=wvb,
                woT=woT,
                bqv=np.ascontiguousarray(bq[gs], dtype=f32),
                bkv=np.ascontiguousarray(bk[gs], dtype=f32),
            )
        )

    in_maps = []
    for c in range(N_CORES):
        b, g = c // G, c % G
        m = dict(qT=qT[b], kT=kT[b], vT=vT[b],
                 onesr=np.ones((1, P), dtype=bf16), **per_group[g])
        in_maps.append(m)
    return in_maps


def _gather(results, bo):
    out = np.zeros((B, S, D), dtype=np.float32)
    for c in range(N_CORES):
        b = c // G
        out[b] += results[c]["out"]
    out += bo.astype(np.float32)
    return out


def run(v, k, q, Wv, bv, Wk, bk, Wq, bq, Wo, bo, trace=False):
    nc = _get_program()
    in_maps = _make_in_maps(v, k, q, Wv, bv, Wk, bk, Wq, bq, Wo, bo)
    res = run_bass_kernel_spmd(
        nc, in_maps, core_ids=list(range(N_CORES)), trace=trace
    )
    return _gather(res.results, np.asarray(bo)), res


def kernel(v, k, q, Wv, bv, Wk, bk, Wq, bq, Wo, bo):
    args = [np.asarray(x, dtype=np.float32)
            for x in (v, k, q, Wv, bv, Wk, bk, Wq, bq, Wo, bo)]
    out, _ = run(*args, trace=bool(int(os.environ.get("MHA_TRACE", "0"))))
    return out
